# revision 13
# baseline (speedup 1.0000x reference)
"""Trainium2 Bass kernel for a 3-layer dual-head GAT (nn_DualHeadGAT).

Strategy (edge-parallel via dst-range sharding):
  - Nodes are split contiguously across 8 cores (6250 each). Edges are
    sorted by destination on the host; each core receives the edges whose
    destination lies in its node range, so every segment reduction
    (softmax denominator, aggregation) is core-local.
  - Per layer: each core computes h' = x_own @ [W | W@As | W@Ad] for its
    own nodes, then an AllGather replicates the feature table to all
    cores. Edge processing gathers table rows by src id (dma_gather),
    computes ea = exp(leaky_relu(es[src] + ed[dst])) and scatter-adds per
    128-node block with a one-hot matmul on the PE:
      U = sum_j M_j^T @ [ea*h | ea],  M[e, n] = (dst_local[e] == n)
    out = relu(U[:, :OC] / U[:, OC:OC+H] + b).
  - Softmax max-subtraction is skipped: alpha is tiny (|alpha| < 1), so
    exp cannot overflow and the result is mathematically identical.
  - dma_gather uses int16 indices, so src gathers are split into
    low (< 32768) and high row halves. ed[dst] is gathered from the
    core-local (pre-AllGather) table with dst-local indices.

Self-contained: hardcodes the problem shapes; host-side preprocessing of
edge_index is pure index manipulation. All float math runs on device.
"""
import math
from contextlib import ExitStack
import numpy as np

import concourse.bass as bass
import concourse.bacc as bacc
import concourse.mybir as mybir
import concourse.tile as tile
from concourse.bass_utils import run_bass_kernel_spmd
from concourse.tile_rust import add_dep_helper

F32 = mybir.dt.float32
F32R = mybir.dt.float32r
I16 = mybir.dt.int16

P = 128
NEG = 0.2
EPS = 1e-9


class Geo:
    def __init__(self, n=50000, ncores=8, split=32768):
        self.N = n
        self.NCORES = ncores
        self.NPD = n // ncores
        self.NBLK = math.ceil(self.NPD / P)
        self.SPLIT = split   # int16 index limit


GEO = Geo()

LAYERS = [(2, 4, 64), (256, 4, 64), (256, 1, 2)]
# gather row size per layer's comb table (64-float multiple)
GC = [320, 320, 64]
# used columns per row (h | es | ed)
CC = [264, 264, 64]

USE_F32R = True
# AllGather output addr space: Shared is faster per docs, but dma_gather from
# a Shared tensor is suspect on this runtime; Local is the safe default.
COMB_SHARED = True
# debug: 0=full, 1=A0+AG0, 2=+2 B0 blocks, 3=+B0+AG1, 4=+B1+AG2
STAGE = 0


# --------------------------------------------------------------------------
# host preprocessing
# --------------------------------------------------------------------------

def _wrap16(seq):
    """dma_gather index layout: idx i lives at [i % 16, i // 16]; tiled to
    128 partitions."""
    seq = np.asarray(seq, np.int16)
    a = seq.reshape(-1, 16).T          # [16, len/16]
    return np.tile(a, (8, 1))          # [128, len/16]


def _host_prep(x, edge_index, weights, geo=GEO):
    N, NCORES, NPD, NBLK, SPLIT = geo.N, geo.NCORES, geo.NPD, geo.NBLK, geo.SPLIT
    src = np.concatenate([np.asarray(edge_index[0]), np.arange(N)]).astype(np.int64)
    dst = np.concatenate([np.asarray(edge_index[1]), np.arange(N)]).astype(np.int64)
    perm = np.argsort(dst, kind="stable")
    s_src = src[perm].astype(np.int32)
    s_dst = dst[perm].astype(np.int32)

    # per (device, block) edge ranges, split into low/high src halves
    starts, stops = [], []
    for d in range(NCORES):
        for k in range(NBLK):
            starts.append(d * NPD + k * P)
            stops.append(min(d * NPD + (k + 1) * P, (d + 1) * NPD))
    e_lo = np.searchsorted(s_dst, starts)
    e_hi = np.searchsorted(s_dst, stops)

    # per-block low/high counts per device
    cnt_lo = np.zeros((NCORES, NBLK), np.int64)
    cnt_hi = np.zeros((NCORES, NBLK), np.int64)
    block_parts = {}
    for d in range(NCORES):
        for k in range(NBLK):
            i = d * NBLK + k
            es = s_src[e_lo[i]:e_hi[i]]
            ed = s_dst[e_lo[i]:e_hi[i]]
            lo_mask = es < SPLIT
            block_parts[(d, k)] = (es[lo_mask], ed[lo_mask],
                                   es[~lo_mask], ed[~lo_mask])
            cnt_lo[d, k] = int(lo_mask.sum())
            cnt_hi[d, k] = int((~lo_mask).sum())

    S_lo = np.maximum(1, np.ceil(cnt_lo.max(axis=0) / P).astype(np.int64))
    S_hi = np.maximum(1, np.ceil(cnt_hi.max(axis=0) / P).astype(np.int64))
    S_all = S_lo + S_hi
    offs = np.concatenate([[0], np.cumsum(S_all)]).astype(np.int64)
    T = int(offs[-1])
    olo = np.concatenate([[0], np.cumsum(S_lo)]).astype(np.int64)
    ohi = np.concatenate([[0], np.cumsum(S_hi)]).astype(np.int64)
    T_lo, T_hi = int(olo[-1]), int(ohi[-1])

    in_maps = []
    x = np.asarray(x, np.float32)
    for d in range(NCORES):
        idx_lo = np.zeros((P, 8 * T_lo), np.int16)
        idx_hi = np.zeros((P, 8 * T_hi), np.int16)
        idx_ed = np.zeros((P, 8 * T), np.int16)
        dst_loc = np.full((P, T), 999.0, np.float32)
        for k in range(NBLK):
            sl, dl_, sh, dh_ = block_parts[(d, k)]
            slo, shi = int(S_lo[k]), int(S_hi[k])
            base = d * NPD + k * P

            buf = np.zeros(slo * P, np.int16)
            buf[:len(sl)] = sl.astype(np.int16)
            idx_lo[:, 8 * int(olo[k]):8 * int(olo[k] + slo)] = _wrap16(buf)

            buf = np.zeros(shi * P, np.int16)
            buf[:len(sh)] = (sh - SPLIT).astype(np.int16)
            idx_hi[:, 8 * int(ohi[k]):8 * int(ohi[k] + shi)] = _wrap16(buf)

            dall = np.concatenate([dl_, dh_])  # wait: placement must match
            # low slots then high slots, each padded independently
            ed_seq = np.zeros((slo + shi) * P, np.int16)
            ed_seq[:len(dl_)] = (dl_ - d * NPD).astype(np.int16)
            ed_seq[slo * P:slo * P + len(dh_)] = (dh_ - d * NPD).astype(np.int16)
            idx_ed[:, 8 * int(offs[k]):8 * int(offs[k] + slo + shi)] = _wrap16(ed_seq)

            loc = np.full((slo + shi) * P, 999.0, np.float32)
            loc[:len(dl_)] = (dl_ - base).astype(np.float32)
            loc[slo * P:slo * P + len(dh_)] = (dh_ - base).astype(np.float32)
            o = int(offs[k])
            dst_loc[:, o:o + slo + shi] = loc.reshape(slo + shi, P).T

        m = {
            "xT": np.ascontiguousarray(x[d * NPD:(d + 1) * NPD].T),
            "idx_lo": idx_lo,
            "idx_hi": idx_hi,
            "idx_ed": idx_ed,
            "dst_local": dst_loc,
            "iota": np.tile(np.arange(P, dtype=np.float32)[None, :], (P, 1)),
            "identity": np.eye(P, dtype=np.float32),
        }
        for li, (W, a_s, a_d, b) in enumerate(weights):
            fin, H, O = LAYERS[li]
            W = np.asarray(W, np.float32)
            a_s = np.asarray(a_s, np.float32)
            a_d = np.asarray(a_d, np.float32)
            b = np.asarray(b, np.float32)
            As = np.zeros((H * O, H), np.float32)
            Ad = np.zeros((H * O, H), np.float32)
            for h in range(H):
                As[h * O:(h + 1) * O, h] = a_s[h]
                Ad[h * O:(h + 1) * O, h] = a_d[h]
            m[f"W{li}"] = W
            m[f"WT{li}"] = np.ascontiguousarray(W.T)
            m[f"As{li}"] = As
            m[f"Ad{li}"] = Ad
            m[f"b{li}"] = np.tile(b[None, :], (P, 1))
        in_maps.append(m)

    plan = {
        "S_lo": tuple(int(s) for s in S_lo),
        "S_hi": tuple(int(s) for s in S_hi),
        "offs": tuple(int(o) for o in offs),
        "olo": tuple(int(o) for o in olo),
        "ohi": tuple(int(o) for o in ohi),
        "T": T, "T_lo": T_lo, "T_hi": T_hi,
    }
    return in_maps, plan


# --------------------------------------------------------------------------
# device program
# --------------------------------------------------------------------------

MM_DT = F32R if USE_F32R else F32


def build_program(plan, geo=GEO):
    N, NCORES, NPD = geo.N, geo.NCORES, geo.NPD
    nc = bacc.Bacc("TRN2", target_bir_lowering=False, debug=False,
                   num_devices=NCORES)

    t_in = {}

    def inp(name, shape, dt=F32):
        t_in[name] = nc.dram_tensor(name, shape, dt, kind="ExternalInput").ap()

    inp("xT", [2, NPD])
    inp("idx_lo", [P, 8 * plan["T_lo"]], I16)
    inp("idx_hi", [P, 8 * plan["T_hi"]], I16)
    inp("idx_ed", [P, 8 * plan["T"]], I16)
    inp("dst_local", [P, plan["T"]])
    inp("iota", [P, P])
    inp("identity", [P, P])
    for li, (fin, H, O) in enumerate(LAYERS):
        OC = H * O
        inp(f"W{li}", [fin, OC])
        inp(f"WT{li}", [OC, fin])
        inp(f"As{li}", [OC, H])
        inp(f"Ad{li}", [OC, H])
        inp(f"b{li}", [P, OC])

    out_own = nc.dram_tensor("out", [NPD, 2], F32, kind="ExternalOutput").ap()

    combs, howns = [], []
    for li in range(3):
        aspace = "Shared" if COMB_SHARED else "Local"
        combs.append(nc.dram_tensor(f"comb{li}", [N, GC[li]], F32,
                                    kind="Internal", addr_space=aspace).ap())
        howns.append(nc.dram_tensor(f"hown{li}", [NPD, GC[li]], F32,
                                    kind="Internal").ap())

    with tile.TileContext(nc) as tc:
        _emit(tc, t_in, out_own, combs, howns, plan, geo)

    nc.compile()
    return nc


def _emit(tc, t_in, out_own, combs, howns, plan, geo=GEO):
    nc = tc.nc
    NCORES, NPD, NBLK, SPLIT = geo.NCORES, geo.NPD, geo.NBLK, geo.SPLIT
    S_lo, S_hi = plan["S_lo"], plan["S_hi"]
    offs, olo, ohi = plan["offs"], plan["olo"], plan["ohi"]
    T, T_lo, T_hi = plan["T"], plan["T_lo"], plan["T_hi"]
    Smax = max(S_lo[k] + S_hi[k] for k in range(NBLK))

    ctx = ExitStack()
    sb_c = ctx.enter_context(tc.tile_pool(name="const", bufs=1))
    sb = ctx.enter_context(tc.tile_pool(name="work", bufs=2))
    sb3 = ctx.enter_context(tc.tile_pool(name="work3", bufs=3))
    ps = ctx.enter_context(tc.tile_pool(name="psum", bufs=2, space="PSUM"))
    ps_u = ctx.enter_context(tc.tile_pool(name="psum_u", bufs=2, space="PSUM"))

    # ---- persistent constants ----
    def load_const(name, shape, dt=F32):
        t = sb_c.tile(shape, dt, tag=name)
        nc.sync.dma_start(out=t[:], in_=t_in[name][:])
        return t

    c_iota = load_const("iota", [P, P])
    c_ident = load_const("identity", [P, P])
    c_ilo = load_const("idx_lo", [P, 8 * T_lo], I16)
    c_ihi = load_const("idx_hi", [P, 8 * T_hi], I16)
    c_ied = load_const("idx_ed", [P, 8 * T], I16)
    c_dloc = load_const("dst_local", [P, T])
    c_xT = load_const("xT", [2, NPD])
    c_b = [load_const(f"b{li}", [P, LAYERS[li][1] * LAYERS[li][2]])
           for li in range(3)]

    # ---- W' = [W | W@As | W@Ad] per layer ----
    wprime = []
    for li, (fin, H, O) in enumerate(LAYERS):
        OC = H * O
        n_fin_t = math.ceil(fin / P)
        n_k_t = math.ceil(OC / P)
        kp = min(P, OC)
        tiles = []
        for fi in range(n_fin_t):
            fr = min(P, fin - fi * P)
            wp = sb_c.tile([P, OC + 2 * H], F32, tag=f"wp{li}_{fi}")
            nc.sync.dma_start(out=wp[:fr, 0:OC],
                              in_=t_in[f"W{li}"][fi * P:fi * P + fr, :])
            for ci, aname in ((0, f"As{li}"), (1, f"Ad{li}")):
                wa_ps = ps.tile([P, H], F32, space="PSUM", tag="wa")
                a_sb = sb.tile([P, n_k_t, H], F32, tag="a_in")
                nc.sync.dma_start(
                    out=a_sb[:kp, 0:n_k_t, :],
                    in_=t_in[aname][:].rearrange("(a p) h -> p a h", p=kp))
                wt_sb = sb.tile([P, n_k_t, P], F32, tag="wt_in")
                nc.sync.dma_start(
                    out=wt_sb[:kp, 0:n_k_t, 0:fr],
                    in_=t_in[f"WT{li}"][:, fi * P:fi * P + fr].rearrange(
                        "(a p) f -> p a f", p=kp))
                for ki in range(n_k_t):
                    kr = min(P, OC - ki * P)
                    nc.tensor.matmul(
                        out=wa_ps[:fr, :],
                        lhsT=wt_sb[:kr, ki, 0:fr],
                        rhs=a_sb[:kr, ki, :],
                        start=(ki == 0), stop=(ki == n_k_t - 1))
                nc.vector.tensor_copy(
                    out=wp[:fr, OC + ci * H:OC + (ci + 1) * H],
                    in_=wa_ps[:fr, :])
            tiles.append(wp)
        wprime.append(tiles)

    # ---- layer 0 phase A ----
    h_writes = []
    for k in range(NBLK):
        nk = min(P, NPD - k * P)
        h_ps = ps.tile([P, CC[0]], F32, space="PSUM", tag="h2ps")
        nc.tensor.matmul(
            out=h_ps[:nk, :],
            lhsT=c_xT[:, k * P:k * P + nk],
            rhs=wprime[0][0][:2, :],
            start=True, stop=True)
        h_sb = sb.tile([P, CC[0]], F32, tag="hsb")
        nc.vector.tensor_copy(out=h_sb[:nk, :], in_=h_ps[:nk, :])
        w = nc.sync.dma_start(out=howns[0][k * P:k * P + nk, 0:CC[0]],
                              in_=h_sb[:nk, :])
        h_writes.append(w)

    # ---- layers ----
    import kernel as _K
    stage = _K.STAGE
    for li in range(3):
        fin, H, O = LAYERS[li]
        OC = H * O
        RC = OC + H
        gc, cc = GC[li], CC[li]
        last = (li == 2)

        ag = nc.gpsimd.collective_compute(
            "AllGather", mybir.AluOpType.bypass,
            replica_groups=[list(range(NCORES))],
            ins=[howns[li][:]], outs=[combs[li][:]],
        )
        for w in h_writes:
            add_dep_helper(ag.ins, w.ins, reason="AG after h writes")
        h_writes = []
        if stage == 1:
            break
        if stage == 3 and li == 1:
            break
        if stage == 4 and li == 2:
            break

        nblk_run = 2 if (stage == 2 and li == 0) else NBLK
        for k in range(nblk_run):
            nk = min(P, NPD - k * P)
            slo, shi = S_lo[k], S_hi[k]
            S = slo + shi
            off = offs[k]

            g = sb.tile([P, Smax, gc], F32, tag="g")
            g1 = nc.gpsimd.dma_gather(
                out_ap=g[:, 0:slo, :], in_ap=combs[li][:],
                idxs_ap=c_ilo[:, 8 * olo[k]:8 * (olo[k] + slo)],
                num_idxs=slo * P, num_idxs_reg=slo * P, elem_size=gc,
                single_packet=False)
            g2 = nc.gpsimd.dma_gather(
                out_ap=g[:, slo:S, :], in_ap=combs[li][SPLIT:, :],
                idxs_ap=c_ihi[:, 8 * ohi[k]:8 * (ohi[k] + shi)],
                num_idxs=shi * P, num_idxs_reg=shi * P, elem_size=gc,
                single_packet=False)
            # ed[dst] from the local table: last 64 used columns of each row
            e = sb.tile([P, Smax, 64], F32, tag="e")
            g3 = nc.gpsimd.dma_gather(
                out_ap=e[:, 0:S, :], in_ap=howns[li][:, cc - 64:cc],
                idxs_ap=c_ied[:, 8 * off:8 * (off + S)],
                num_idxs=S * P, num_idxs_reg=S * P, elem_size=64, elem_step=gc,
                single_packet=False)
            for gi in (g1, g2, g3):
                add_dep_helper(gi.ins, ag.ins, reason="gather after AG")

            # column offsets inside gathered rows
            if not last:
                es_sl = g[:, 0:S, 256:260]
                ed_sl = e[:, 0:S, 60:64]
                h_sl = g[:, 0:S, 0:OC]
            else:
                es_sl = g[:, 0:S, 2:3]
                ed_sl = e[:, 0:S, 3:4]
                h_sl = g[:, 0:S, 0:2]

            al = sb.tile([P, Smax, H], F32, tag="al")
            nc.vector.tensor_tensor(out=al[:, 0:S, :], in0=es_sl, in1=ed_sl,
                                    op=mybir.AluOpType.add)
            al2 = sb.tile([P, Smax, H], F32, tag="al2")
            nc.vector.tensor_scalar_mul(out=al2[:, 0:S, :], in0=al[:, 0:S, :],
                                        scalar1=NEG)
            al3 = sb.tile([P, Smax, H], F32, tag="al3")
            nc.vector.tensor_tensor(out=al3[:, 0:S, :], in0=al[:, 0:S, :],
                                    in1=al2[:, 0:S, :], op=mybir.AluOpType.max)
            ea = sb.tile([P, Smax, H], F32, tag="ea")
            nc.scalar.activation(out=ea[:, 0:S, :], in_=al3[:, 0:S, :],
                                 func=mybir.ActivationFunctionType.Exp)

            mmdt = MM_DT if not last else F32
            m = sb.tile([P, Smax * P], mmdt, tag="m")
            mv = m[:].rearrange("p (s n) -> p s n", n=P)
            nc.vector.tensor_tensor(
                out=mv[:, 0:S, :],
                in0=c_dloc[:, off:off + S].unsqueeze(2).to_broadcast([P, S, P]),
                in1=c_iota[:].unsqueeze(1).to_broadcast([P, S, P]),
                op=mybir.AluOpType.is_equal)

            rhs = sb.tile([P, Smax, RC], mmdt, tag="rhs")
            nc.vector.tensor_tensor(
                out=rhs[:, 0:S, 0:OC].rearrange("p s (h o) -> p s h o", o=O),
                in0=h_sl.rearrange("p s (h o) -> p s h o", o=O),
                in1=ea[:, 0:S, :].unsqueeze(3).to_broadcast([P, S, H, O]),
                op=mybir.AluOpType.mult)
            nc.vector.tensor_copy(out=rhs[:, 0:S, OC:RC], in_=ea[:, 0:S, :])

            u_ps = ps_u.tile([P, RC], F32, space="PSUM", tag="u")
            for j in range(S):
                nc.tensor.matmul(
                    out=u_ps[:],
                    lhsT=m[:, j * P:(j + 1) * P],
                    rhs=rhs[:, j, :],
                    start=(j == 0), stop=(j == S - 1))

            den = sb.tile([P, H], F32, tag="den")
            nc.vector.tensor_scalar_add(out=den[:], in0=u_ps[:, OC:RC],
                                        scalar1=EPS)
            rec = sb.tile([P, H], F32, tag="rec")
            nc.vector.reciprocal(out=rec[:], in_=den[:])
            ob = sb.tile([P, OC], F32, tag="ob")
            nc.vector.tensor_tensor(
                out=ob[:].rearrange("p (h o) -> p h o", o=O),
                in0=u_ps[:, 0:OC].rearrange("p (h o) -> p h o", o=O),
                in1=rec[:].unsqueeze(2).to_broadcast([P, H, O]),
                op=mybir.AluOpType.mult)
            ob2 = sb.tile([P, OC], F32, tag="ob2")
            nc.vector.tensor_tensor(out=ob2[:], in0=ob[:], in1=c_b[li][:],
                                    op=mybir.AluOpType.add)
            orl = sb.tile([P, OC], F32, tag="orl")
            nc.scalar.activation(out=orl[:], in_=ob2[:],
                                 func=mybir.ActivationFunctionType.Relu)

            if last:
                nc.sync.dma_start(out=out_own[k * P:k * P + nk, :],
                                  in_=orl[:nk, 0:2])
            else:
                fin2, H2, O2 = LAYERS[li + 1]
                cc2 = H2 * O2 + 2 * H2
                h2_ps = ps.tile([P, cc2], F32, space="PSUM", tag="h2ps")
                nf = OC // P
                for f in range(nf):
                    tp_ps = ps.tile([P, P], F32, space="PSUM", tag="tp")
                    nc.tensor.transpose(
                        out=tp_ps[:], in_=orl[:, f * P:(f + 1) * P],
                        identity=c_ident[:])
                    xt = sb3.tile([P, P], F32, tag="xt")
                    nc.vector.tensor_copy(out=xt[:], in_=tp_ps[:])
                    nc.tensor.matmul(
                        out=h2_ps[:], lhsT=xt[:], rhs=wprime[li + 1][f][:, :],
                        start=(f == 0), stop=(f == nf - 1))
                h2_sb = sb.tile([P, cc2], F32, tag="h2sb")
                nc.vector.tensor_copy(out=h2_sb[:nk, :], in_=h2_ps[:nk, :])
                w = nc.sync.dma_start(
                    out=howns[li + 1][k * P:k * P + nk, 0:cc2],
                    in_=h2_sb[:nk, :])
                h_writes.append(w)
        if stage == 2 and li == 0:
            break

    ctx.close()


# --------------------------------------------------------------------------
# entry point
# --------------------------------------------------------------------------

_cache = {}
TRACE = False
last_result = None


def kernel(x, edge_index, W0, a_src0, a_dst0, b0, W1, a_src1, a_dst1, b1,
           W2, a_src2, a_dst2, b2):
    weights = [(W0, a_src0, a_dst0, b0), (W1, a_src1, a_dst1, b1),
               (W2, a_src2, a_dst2, b2)]
    in_maps, plan = _host_prep(np.asarray(x), np.asarray(edge_index), weights)

    key = (plan["S_lo"], plan["S_hi"])
    if key not in _cache:
        _cache[key] = build_program(plan)
    nc = _cache[key]

    global last_result
    res = run_bass_kernel_spmd(nc, in_maps, core_ids=list(range(GEO.NCORES)),
                               trace=TRACE)
    last_result = res
    out = np.concatenate(
        [res.results[d]["out"] for d in range(GEO.NCORES)], axis=0)
    return out.astype(np.float32)


# revision 19
# speedup vs baseline: 343.6963x; 343.6963x over previous
"""Trainium2 Bass kernel for a 3-layer dual-head GAT (nn_DualHeadGAT).

Strategy (edge-parallel via dst-range sharding):
  - Nodes are split contiguously across 8 cores (6250 each). Edges are
    sorted by destination on the host; each core receives the edges whose
    destination lies in its node range, so every segment reduction
    (softmax denominator, aggregation) is core-local.
  - Per layer: each core computes h' = x_own @ [W | W@As | W@Ad] for its
    own nodes, then an AllGather replicates the feature table to all
    cores. Edge processing gathers table rows by src id (dma_gather),
    computes ea = exp(leaky_relu(es[src] + ed[dst])) and scatter-adds per
    128-node block with a one-hot matmul on the PE:
      U = sum_j M_j^T @ [ea*h | ea],  M[e, n] = (dst_local[e] == n)
    out = relu(U[:, :OC] / U[:, OC:OC+H] + b).
  - Softmax max-subtraction is skipped: alpha is tiny (|alpha| < 1), so
    exp cannot overflow and the result is mathematically identical.
  - dma_gather uses int16 indices, so src gathers are split into
    low (< 32768) and high row halves. ed[dst] is gathered from the
    core-local (pre-AllGather) table with dst-local indices.

Self-contained: hardcodes the problem shapes; host-side preprocessing of
edge_index is pure index manipulation. All float math runs on device.
"""
import math
from contextlib import ExitStack
import numpy as np

import concourse.bass as bass
import concourse.bacc as bacc
import concourse.mybir as mybir
import concourse.tile as tile
from concourse.bass_utils import run_bass_kernel_spmd
from concourse.tile_rust import add_dep_helper

F32 = mybir.dt.float32
F32R = mybir.dt.float32r
I16 = mybir.dt.int16

P = 128
NEG = 0.2
EPS = 1e-9


class Geo:
    def __init__(self, n=50000, ncores=8, split=32768):
        self.N = n
        self.NCORES = ncores
        self.NPD = n // ncores
        self.NBLK = math.ceil(self.NPD / P)
        self.SPLIT = split   # int16 index limit


GEO = Geo()

LAYERS = [(2, 4, 64), (256, 4, 64), (256, 1, 2)]
# gather row size per layer's comb table (64-float multiple)
GC = [320, 320, 64]
# used columns per row (h | es | ed)
CC = [264, 264, 64]

USE_F32R = True
# AllGather output addr space: Shared is faster per docs, but dma_gather from
# a Shared tensor is suspect on this runtime; Local is the safe default.
COMB_SHARED = True
# debug: 0=full, 1=A0+AG0, 2=+2 B0 blocks, 3=+B0+AG1, 4=+B1+AG2
STAGE = 0


# --------------------------------------------------------------------------
# host preprocessing
# --------------------------------------------------------------------------

def _wrap16(seq):
    """dma_gather index layout: idx i lives at [i % 16, i // 16]; tiled to
    128 partitions."""
    seq = np.asarray(seq, np.int16)
    a = seq.reshape(-1, 16).T          # [16, len/16]
    return np.tile(a, (8, 1))          # [128, len/16]


def _host_prep(x, edge_index, weights, geo=GEO):
    N, NCORES, NPD, NBLK, SPLIT = geo.N, geo.NCORES, geo.NPD, geo.NBLK, geo.SPLIT
    # self-loops are handled analytically per block (own rows are local);
    # only real edges go through the gather pipeline
    src = np.asarray(edge_index[0]).astype(np.int64)
    dst = np.asarray(edge_index[1]).astype(np.int64)
    perm = np.argsort(dst, kind="stable")
    s_src = src[perm].astype(np.int32)
    s_dst = dst[perm].astype(np.int32)

    # per (device, block) edge ranges, split into low/high src halves
    starts, stops = [], []
    for d in range(NCORES):
        for k in range(NBLK):
            starts.append(d * NPD + k * P)
            stops.append(min(d * NPD + (k + 1) * P, (d + 1) * NPD))
    e_lo = np.searchsorted(s_dst, starts)
    e_hi = np.searchsorted(s_dst, stops)

    # per-block low/high counts per device
    cnt_lo = np.zeros((NCORES, NBLK), np.int64)
    cnt_hi = np.zeros((NCORES, NBLK), np.int64)
    block_parts = {}
    for d in range(NCORES):
        for k in range(NBLK):
            i = d * NBLK + k
            es = s_src[e_lo[i]:e_hi[i]]
            ed = s_dst[e_lo[i]:e_hi[i]]
            lo_mask = es < SPLIT
            block_parts[(d, k)] = (es[lo_mask], ed[lo_mask],
                                   es[~lo_mask], ed[~lo_mask])
            cnt_lo[d, k] = int(lo_mask.sum())
            cnt_hi[d, k] = int((~lo_mask).sum())

    S_lo = np.maximum(1, np.ceil(cnt_lo.max(axis=0) / P).astype(np.int64))
    S_hi = np.maximum(1, np.ceil(cnt_hi.max(axis=0) / P).astype(np.int64))
    S_all = S_lo + S_hi
    offs = np.concatenate([[0], np.cumsum(S_all)]).astype(np.int64)
    T = int(offs[-1])
    olo = np.concatenate([[0], np.cumsum(S_lo)]).astype(np.int64)
    ohi = np.concatenate([[0], np.cumsum(S_hi)]).astype(np.int64)
    T_lo, T_hi = int(olo[-1]), int(ohi[-1])

    in_maps = []
    x = np.asarray(x, np.float32)
    for d in range(NCORES):
        idx_lo = np.zeros((P, 8 * T_lo), np.int16)
        idx_hi = np.zeros((P, 8 * T_hi), np.int16)
        idx_ed = np.zeros((P, 8 * T), np.int16)
        dst_loc = np.full((P, T), 999.0, np.float32)
        for k in range(NBLK):
            sl, dl_, sh, dh_ = block_parts[(d, k)]
            slo, shi = int(S_lo[k]), int(S_hi[k])
            base = d * NPD + k * P

            buf = np.zeros(slo * P, np.int16)
            buf[:len(sl)] = sl.astype(np.int16)
            idx_lo[:, 8 * int(olo[k]):8 * int(olo[k] + slo)] = _wrap16(buf)

            buf = np.zeros(shi * P, np.int16)
            buf[:len(sh)] = (sh - SPLIT).astype(np.int16)
            idx_hi[:, 8 * int(ohi[k]):8 * int(ohi[k] + shi)] = _wrap16(buf)

            dall = np.concatenate([dl_, dh_])  # wait: placement must match
            # low slots then high slots, each padded independently
            ed_seq = np.zeros((slo + shi) * P, np.int16)
            ed_seq[:len(dl_)] = (dl_ - d * NPD).astype(np.int16)
            ed_seq[slo * P:slo * P + len(dh_)] = (dh_ - d * NPD).astype(np.int16)
            idx_ed[:, 8 * int(offs[k]):8 * int(offs[k] + slo + shi)] = _wrap16(ed_seq)

            loc = np.full((slo + shi) * P, 999.0, np.float32)
            loc[:len(dl_)] = (dl_ - base).astype(np.float32)
            loc[slo * P:slo * P + len(dh_)] = (dh_ - base).astype(np.float32)
            o = int(offs[k])
            dst_loc[:, o:o + slo + shi] = loc.reshape(slo + shi, P).T

        m = {
            "xT": np.ascontiguousarray(x[d * NPD:(d + 1) * NPD].T),
            "idx_lo": idx_lo,
            "idx_hi": idx_hi,
            "idx_ed": idx_ed,
            "dst_local": dst_loc,
            "iota": np.tile(np.arange(P, dtype=np.float32)[None, :], (P, 1)),
            "identity": np.eye(P, dtype=np.float32),
        }
        for li, (W, a_s, a_d, b) in enumerate(weights):
            fin, H, O = LAYERS[li]
            W = np.asarray(W, np.float32)
            a_s = np.asarray(a_s, np.float32)
            a_d = np.asarray(a_d, np.float32)
            b = np.asarray(b, np.float32)
            As = np.zeros((H * O, H), np.float32)
            Ad = np.zeros((H * O, H), np.float32)
            for h in range(H):
                As[h * O:(h + 1) * O, h] = a_s[h]
                Ad[h * O:(h + 1) * O, h] = a_d[h]
            m[f"W{li}"] = W
            m[f"WT{li}"] = np.ascontiguousarray(W.T)
            m[f"As{li}"] = As
            m[f"Ad{li}"] = Ad
            m[f"b{li}"] = np.tile(b[None, :], (P, 1))
        in_maps.append(m)

    plan = {
        "S_lo": tuple(int(s) for s in S_lo),
        "S_hi": tuple(int(s) for s in S_hi),
        "offs": tuple(int(o) for o in offs),
        "olo": tuple(int(o) for o in olo),
        "ohi": tuple(int(o) for o in ohi),
        "T": T, "T_lo": T_lo, "T_hi": T_hi,
    }
    return in_maps, plan


# --------------------------------------------------------------------------
# device program
# --------------------------------------------------------------------------

MM_DT = F32R if USE_F32R else F32


def build_program(plan, geo=GEO):
    N, NCORES, NPD = geo.N, geo.NCORES, geo.NPD
    nc = bacc.Bacc("TRN2", target_bir_lowering=False, debug=False,
                   num_devices=NCORES, num_swdge_queues=4)

    t_in = {}

    def inp(name, shape, dt=F32):
        t_in[name] = nc.dram_tensor(name, shape, dt, kind="ExternalInput").ap()

    inp("xT", [2, NPD])
    inp("idx_lo", [P, 8 * plan["T_lo"]], I16)
    inp("idx_hi", [P, 8 * plan["T_hi"]], I16)
    inp("idx_ed", [P, 8 * plan["T"]], I16)
    inp("dst_local", [P, plan["T"]])
    inp("iota", [P, P])
    inp("identity", [P, P])
    for li, (fin, H, O) in enumerate(LAYERS):
        OC = H * O
        inp(f"W{li}", [fin, OC])
        inp(f"WT{li}", [OC, fin])
        inp(f"As{li}", [OC, H])
        inp(f"Ad{li}", [OC, H])
        inp(f"b{li}", [P, OC])

    out_own = nc.dram_tensor("out", [NPD, 2], F32, kind="ExternalOutput").ap()

    combs, howns = [], []
    for li in range(3):
        aspace = "Shared" if COMB_SHARED else "Local"
        combs.append(nc.dram_tensor(f"comb{li}", [N, GC[li]], F32,
                                    kind="Internal", addr_space=aspace).ap())
        howns.append(nc.dram_tensor(f"hown{li}", [NPD, GC[li]], F32,
                                    kind="Internal").ap())

    with tile.TileContext(nc) as tc:
        _emit(tc, t_in, out_own, combs, howns, plan, geo)

    nc.compile()
    return nc


def _emit(tc, t_in, out_own, combs, howns, plan, geo=GEO):
    _emit._qi = 0
    nc = tc.nc
    NCORES, NPD, NBLK, SPLIT = geo.NCORES, geo.NPD, geo.NBLK, geo.SPLIT
    S_lo, S_hi = plan["S_lo"], plan["S_hi"]
    offs, olo, ohi = plan["offs"], plan["olo"], plan["ohi"]
    T, T_lo, T_hi = plan["T"], plan["T_lo"], plan["T_hi"]
    Smax = max(S_lo[k] + S_hi[k] for k in range(NBLK))

    ctx = ExitStack()
    sb_c = ctx.enter_context(tc.tile_pool(name="const", bufs=1))
    sb = ctx.enter_context(tc.tile_pool(name="work", bufs=2))
    sb3 = ctx.enter_context(tc.tile_pool(name="work3", bufs=3))
    ps = ctx.enter_context(tc.tile_pool(name="psum", bufs=2, space="PSUM"))
    ps_u = ctx.enter_context(tc.tile_pool(name="psum_u", bufs=2, space="PSUM"))

    # ---- persistent constants ----
    def load_const(name, shape, dt=F32):
        t = sb_c.tile(shape, dt, tag=name)
        nc.sync.dma_start(out=t[:], in_=t_in[name][:])
        return t

    c_iota = load_const("iota", [P, P])
    c_ident = load_const("identity", [P, P])
    c_ilo = load_const("idx_lo", [P, 8 * T_lo], I16)
    c_ihi = load_const("idx_hi", [P, 8 * T_hi], I16)
    c_ied = load_const("idx_ed", [P, 8 * T], I16)
    c_dloc = load_const("dst_local", [P, T])
    c_xT = load_const("xT", [2, NPD])
    c_b = [load_const(f"b{li}", [P, LAYERS[li][1] * LAYERS[li][2]])
           for li in range(3)]

    # ---- W' = [W | W@As | W@Ad] per layer ----
    wprime = []
    for li, (fin, H, O) in enumerate(LAYERS):
        OC = H * O
        n_fin_t = math.ceil(fin / P)
        n_k_t = math.ceil(OC / P)
        kp = min(P, OC)
        tiles = []
        for fi in range(n_fin_t):
            fr = min(P, fin - fi * P)
            wp = sb_c.tile([P, OC + 2 * H], F32, tag=f"wp{li}_{fi}")
            nc.sync.dma_start(out=wp[:fr, 0:OC],
                              in_=t_in[f"W{li}"][fi * P:fi * P + fr, :])
            for ci, aname in ((0, f"As{li}"), (1, f"Ad{li}")):
                wa_ps = ps.tile([P, H], F32, space="PSUM", tag="wa")
                a_sb = sb.tile([P, n_k_t, H], F32, tag="a_in")
                nc.sync.dma_start(
                    out=a_sb[:kp, 0:n_k_t, :],
                    in_=t_in[aname][:].rearrange("(a p) h -> p a h", p=kp))
                wt_sb = sb.tile([P, n_k_t, P], F32, tag="wt_in")
                nc.sync.dma_start(
                    out=wt_sb[:kp, 0:n_k_t, 0:fr],
                    in_=t_in[f"WT{li}"][:, fi * P:fi * P + fr].rearrange(
                        "(a p) f -> p a f", p=kp))
                for ki in range(n_k_t):
                    kr = min(P, OC - ki * P)
                    nc.tensor.matmul(
                        out=wa_ps[:fr, :],
                        lhsT=wt_sb[:kr, ki, 0:fr],
                        rhs=a_sb[:kr, ki, :],
                        start=(ki == 0), stop=(ki == n_k_t - 1))
                nc.vector.tensor_copy(
                    out=wp[:fr, OC + ci * H:OC + (ci + 1) * H],
                    in_=wa_ps[:fr, :])
            tiles.append(wp)
        wprime.append(tiles)

    # ---- layer 0 phase A ----
    h_writes = []
    for k in range(NBLK):
        nk = min(P, NPD - k * P)
        h_ps = ps.tile([P, CC[0]], F32, space="PSUM", tag="h2ps")
        nc.tensor.matmul(
            out=h_ps[:nk, :],
            lhsT=c_xT[:, k * P:k * P + nk],
            rhs=wprime[0][0][:2, :],
            start=True, stop=True)
        h_sb = sb.tile([P, CC[0]], F32, tag="hsb")
        nc.vector.tensor_copy(out=h_sb[:nk, :], in_=h_ps[:nk, :])
        w = nc.sync.dma_start(out=howns[0][k * P:k * P + nk, 0:CC[0]],
                              in_=h_sb[:nk, :])
        h_writes.append(w)

    # ---- layers ----
    import kernel as _K
    stage = _K.STAGE
    for li in range(3):
        fin, H, O = LAYERS[li]
        OC = H * O
        RC = OC + H
        gc, cc = GC[li], CC[li]
        last = (li == 2)

        if NCORES > 1:
            ag = nc.gpsimd.collective_compute(
                "AllGather", mybir.AluOpType.bypass,
                replica_groups=[list(range(NCORES))],
                ins=[howns[li][:]], outs=[combs[li][:]],
            )
        else:
            nfull = (NPD // P) * P
            nc.sync.dma_start(
                out=combs[li][0:nfull, :].rearrange("(a p) c -> p a c", p=P),
                in_=howns[li][0:nfull, :].rearrange("(a p) c -> p a c", p=P))
            ag = nc.sync.dma_start(
                out=combs[li][nfull:NPD, :],
                in_=howns[li][nfull:NPD, :])
        for w in h_writes:
            add_dep_helper(ag.ins, w.ins, reason="AG after h writes")
        prev_writes = h_writes
        h_writes = []
        if stage == 1:
            break
        if stage == 3 and li == 1:
            break
        if stage == 4 and li == 2:
            break

        nblk_run = 2 if (stage == 2 and li == 0) else NBLK
        for k in range(nblk_run):
            nk = min(P, NPD - k * P)
            slo, shi = S_lo[k], S_hi[k]
            S = slo + shi
            off = offs[k]

            # split gathers into <=1024-descriptor calls (single-packet
            # SWDGE path tops out at 64 descriptors x 16 engines per call)
            CH = 8

            def _gather(out3, o0, in_ap, idxt, ioff, cnt, elem, estep=None,
                        dep=None):
                for c0 in range(0, cnt, CH):
                    cs = min(CH, cnt - c0)
                    qn = (_emit._qi // 2) % 4
                    _emit._qi += 1
                    gi = nc.gpsimd.dma_gather(
                        out_ap=out3[:, o0 + c0:o0 + c0 + cs, :], in_ap=in_ap,
                        idxs_ap=idxt[:, 8 * (ioff + c0):8 * (ioff + c0 + cs)],
                        num_idxs=cs * P, num_idxs_reg=cs * P,
                        elem_size=elem, elem_step=estep, queue_num=qn)
                    add_dep_helper(gi.ins, (dep or ag).ins,
                                   reason="gather after producer")

            g = sb.tile([P, Smax, gc], F32, tag="g")
            _gather(g, 0, combs[li][:], c_ilo, olo[k], slo, gc)
            _gather(g, slo, combs[li][SPLIT:, :], c_ihi, ohi[k], shi, gc)
            # ed[dst] from the local table: last 64 used columns of each row
            e = sb.tile([P, Smax, 64], F32, tag="e")
            ed_dep = prev_writes[k] if k < len(prev_writes) else None
            _gather(e, 0, howns[li][:, cc - 64:cc], c_ied, off, S, 64, gc,
                    dep=ed_dep)

            # column offsets inside gathered rows
            if not last:
                es_sl = g[:, 0:S, 256:260]
                ed_sl = e[:, 0:S, 60:64]
                h_sl = g[:, 0:S, 0:OC]
            else:
                es_sl = g[:, 0:S, 2:3]
                ed_sl = e[:, 0:S, 3:4]
                h_sl = g[:, 0:S, 0:2]

            al = sb.tile([P, Smax, H], F32, tag="al")
            nc.vector.tensor_tensor(out=al[:, 0:S, :], in0=es_sl, in1=ed_sl,
                                    op=mybir.AluOpType.add)
            al2 = sb.tile([P, Smax, H], F32, tag="al2")
            nc.vector.tensor_scalar_mul(out=al2[:, 0:S, :], in0=al[:, 0:S, :],
                                        scalar1=NEG)
            al3 = sb.tile([P, Smax, H], F32, tag="al3")
            nc.vector.tensor_tensor(out=al3[:, 0:S, :], in0=al[:, 0:S, :],
                                    in1=al2[:, 0:S, :], op=mybir.AluOpType.max)
            ea = sb.tile([P, Smax, H], F32, tag="ea")
            nc.scalar.activation(out=ea[:, 0:S, :], in_=al3[:, 0:S, :],
                                 func=mybir.ActivationFunctionType.Exp)

            mmdt = MM_DT if not last else F32
            m = sb.tile([P, Smax * P], mmdt, tag="m")
            mv = m[:].rearrange("p (s n) -> p s n", n=P)
            nc.vector.tensor_tensor(
                out=mv[:, 0:S, :],
                in0=c_dloc[:, off:off + S].unsqueeze(2).to_broadcast([P, S, P]),
                in1=c_iota[:].unsqueeze(1).to_broadcast([P, S, P]),
                op=mybir.AluOpType.is_equal)

            rhs = sb.tile([P, Smax, RC], mmdt, tag="rhs")
            nc.vector.tensor_tensor(
                out=rhs[:, 0:S, 0:OC].rearrange("p s (h o) -> p s h o", o=O),
                in0=h_sl.rearrange("p s (h o) -> p s h o", o=O),
                in1=ea[:, 0:S, :].unsqueeze(3).to_broadcast([P, S, H, O]),
                op=mybir.AluOpType.mult)
            nc.vector.tensor_copy(out=rhs[:, 0:S, OC:RC], in_=ea[:, 0:S, :])

            u_ps = ps_u.tile([P, RC], F32, space="PSUM", tag="u")
            for j in range(S):
                nc.tensor.matmul(
                    out=u_ps[:],
                    lhsT=m[:, j * P:(j + 1) * P],
                    rhs=rhs[:, j, :],
                    start=(j == 0), stop=(j == S - 1))

            # self-loop: own rows are contiguous in the local table
            hb = sb.tile([P, 2 * H + OC], F32, tag="hb")
            wsl = nc.sync.dma_start(out=hb[:nk, :],
                                    in_=howns[li][k * P:k * P + nk, 0:OC + 2 * H])
            if k < len(prev_writes):
                add_dep_helper(wsl.ins, prev_writes[k].ins,
                               reason="self rows after h write")
            asl = sb.tile([P, H], F32, tag="asl")
            nc.vector.tensor_tensor(out=asl[:], in0=hb[:, OC:OC + H],
                                    in1=hb[:, OC + H:OC + 2 * H],
                                    op=mybir.AluOpType.add)
            asl2 = sb.tile([P, H], F32, tag="asl2")
            nc.vector.tensor_scalar_mul(out=asl2[:], in0=asl[:], scalar1=NEG)
            asl3 = sb.tile([P, H], F32, tag="asl3")
            nc.vector.tensor_tensor(out=asl3[:], in0=asl[:], in1=asl2[:],
                                    op=mybir.AluOpType.max)
            easl = sb.tile([P, H], F32, tag="easl")
            nc.scalar.activation(out=easl[:], in_=asl3[:],
                                 func=mybir.ActivationFunctionType.Exp)
            den = sb.tile([P, H], F32, tag="den")
            nc.vector.tensor_tensor(out=den[:], in0=u_ps[:, OC:RC],
                                    in1=easl[:], op=mybir.AluOpType.add)
            rec = sb.tile([P, H], F32, tag="rec")
            nc.vector.reciprocal(out=rec[:], in_=den[:])
            smsg = sb.tile([P, OC], F32, tag="smsg")
            nc.vector.tensor_tensor(
                out=smsg[:].rearrange("p (h o) -> p h o", o=O),
                in0=hb[:, 0:OC].rearrange("p (h o) -> p h o", o=O),
                in1=easl[:].unsqueeze(2).to_broadcast([P, H, O]),
                op=mybir.AluOpType.mult)
            u2 = sb.tile([P, OC], F32, tag="u2")
            nc.vector.tensor_tensor(out=u2[:], in0=u_ps[:, 0:OC], in1=smsg[:],
                                    op=mybir.AluOpType.add)
            ob = sb.tile([P, OC], F32, tag="ob")
            nc.vector.tensor_tensor(
                out=ob[:].rearrange("p (h o) -> p h o", o=O),
                in0=u2[:].rearrange("p (h o) -> p h o", o=O),
                in1=rec[:].unsqueeze(2).to_broadcast([P, H, O]),
                op=mybir.AluOpType.mult)
            ob2 = sb.tile([P, OC], F32, tag="ob2")
            nc.vector.tensor_tensor(out=ob2[:], in0=ob[:], in1=c_b[li][:],
                                    op=mybir.AluOpType.add)
            orl = sb.tile([P, OC], F32, tag="orl")
            nc.scalar.activation(out=orl[:], in_=ob2[:],
                                 func=mybir.ActivationFunctionType.Relu)

            if last:
                nc.sync.dma_start(out=out_own[k * P:k * P + nk, :],
                                  in_=orl[:nk, 0:2])
            else:
                fin2, H2, O2 = LAYERS[li + 1]
                cc2 = H2 * O2 + 2 * H2
                h2_ps = ps.tile([P, cc2], F32, space="PSUM", tag="h2ps")
                nf = OC // P
                for f in range(nf):
                    tp_ps = ps.tile([P, P], F32, space="PSUM", tag="tp")
                    nc.tensor.transpose(
                        out=tp_ps[:], in_=orl[:, f * P:(f + 1) * P],
                        identity=c_ident[:])
                    xt = sb3.tile([P, P], F32, tag="xt")
                    nc.vector.tensor_copy(out=xt[:], in_=tp_ps[:])
                    nc.tensor.matmul(
                        out=h2_ps[:], lhsT=xt[:], rhs=wprime[li + 1][f][:, :],
                        start=(f == 0), stop=(f == nf - 1))
                h2_sb = sb.tile([P, cc2], F32, tag="h2sb")
                nc.vector.tensor_copy(out=h2_sb[:nk, :], in_=h2_ps[:nk, :])
                w = nc.sync.dma_start(
                    out=howns[li + 1][k * P:k * P + nk, 0:cc2],
                    in_=h2_sb[:nk, :])
                h_writes.append(w)
        if stage == 2 and li == 0:
            break

    ctx.close()


# --------------------------------------------------------------------------
# entry point
# --------------------------------------------------------------------------

_cache = {}
TRACE = False
last_result = None


def kernel(x, edge_index, W0, a_src0, a_dst0, b0, W1, a_src1, a_dst1, b1,
           W2, a_src2, a_dst2, b2):
    weights = [(W0, a_src0, a_dst0, b0), (W1, a_src1, a_dst1, b1),
               (W2, a_src2, a_dst2, b2)]
    in_maps, plan = _host_prep(np.asarray(x), np.asarray(edge_index), weights)

    key = (plan["S_lo"], plan["S_hi"])
    if key not in _cache:
        _cache[key] = build_program(plan)
    nc = _cache[key]

    global last_result
    res = run_bass_kernel_spmd(nc, in_maps, core_ids=list(range(GEO.NCORES)),
                               trace=TRACE)
    last_result = res
    out = np.concatenate(
        [res.results[d]["out"] for d in range(GEO.NCORES)], axis=0)
    return out.astype(np.float32)
